# revision 1
# baseline (speedup 1.0000x reference)
"""Compact Bilinear Pooling (B=16, C=512, HW=196, OUT=8192) on 8 TRN2 NeuronCores.

Math (as the original baseline): per batch, cbp = irfft(rfft(p1)*rfft(p2))*OUT
with p_j = x_hw @ sketch_j, summed over the 196 spatial positions, then
signed-sqrt + L2-normalize. Count-sketch rows have one +-1 entry, so
U_j = rfft(p_j) = x @ A_j with A_j a phase table; the spatial sum moves
inside the transform and one 64x128 Cooley-Tukey irfft per batch finishes.
fp16 hi/lo pairs for x and the tables keep the signed-sqrt tail (~150x error
amplification near zero) inside the accuracy budget.

Perf: per-run time is dominated by host<->device I/O handling, not compute
(measured: the whole compute pipeline hides under the fixed I/O cost). So:
  - Nothing big is shipped. The 34.6MB of fp16-pair phase tables are
    generated ON DEVICE per 512-column f-tile, just in time, double
    buffered: the base tile comes from an exact integer-phase pipeline
    (ph = (((hq*f) mod 128) << 6) + hr*f + off via two tiny f16 matmuls,
    int ALU ops, and the Sin activation); tiles t=1..8 multiply the base by
    diagonal complex ratios exp(-2pi i 512 t h / N) as fp16 diagonal
    matmuls with 3-term pair products accumulating in PSUM f32.
  - The irfft DFT matrices, band/one-hot/iota/ones constants are all built
    on device with the same machinery.
  - ONE input dram tensor (x-shard + ~90KB of f16 diag/meta words): input
    tensor count and bytes both carry a per-run cost.
  - Output is int8-quantized on device with a per-batch f32 scale packed
    into spare columns (output bytes cost ~10x more per byte than input
    bytes); the host only dequantizes/reformats.
  - The main loop processes chunk pairs (N=512 matmuls, u1/u2 spectra in
    separate PSUM banks) to halve instruction count.

Sharding: data-parallel over batch, 2 batches per core, no collectives.
"""

import numpy as np

B, C, HW, N = 16, 512, 196, 8192
NF = N // 2 + 1          # 4097 rfft bins
CHUNK = 128              # frequency columns per main-loop chunk
FT = 512                 # frequency columns per generated f-tile
NT = 9                   # f-tiles (4608 cols >= 4097)
NCHUNK = 32              # regular chunks (f < 4096); chunk 32 = Nyquist
NCORES = 8
BPC = B // NCORES
EPS_SQRT = 1e-5
EPS_NORM = 1e-12

_COMPILED = {}

# ---------------------------------------------------------------- host side

# xpack f32 layout: x | diag (f16 words) | metaA f16 words | metaB f16 words
_XP_X = 0
_XP_DIAG = BPC * C * HW              # [128, 128] f32 words = [128, 256] f16
_XP_METAA = _XP_DIAG + 128 * 128     # [2, 1408] f32 words  = [2, 2816] f16
_XP_METAB = _XP_METAA + 2 * 1408     # [1, 1408] f32 words  = [1, 2816] f16
_XP_TOT = _XP_METAB + 1408


def _extract(sk):
    sk = np.asarray(sk)
    h = np.abs(sk).argmax(axis=1).astype(np.int64)
    s = sk[np.arange(C), h]
    return h, (s < 0).astype(np.int64)


def _diag_words(h1, h2):
    dg = np.zeros((128, 256), np.float16)
    for sk, h in ((0, h1), (1, h2)):
        for cc in range(4):
            hc = h[cc * 128:(cc + 1) * 128].astype(np.float64)
            for t in range(1, 9):
                a = 2 * np.pi * ((512 * t * hc) % N) / N
                cr, sn = np.cos(a), np.sin(a)
                crh = cr.astype(np.float16)
                snh = sn.astype(np.float16)
                base = ((sk * 4 + cc) * 8 + (t - 1)) * 4
                dg[:, base + 0] = crh
                dg[:, base + 1] = (cr - crh).astype(np.float16)
                dg[:, base + 2] = snh
                dg[:, base + 3] = (sn - snh).astype(np.float16)
    return dg.view(np.float32)


def _meta_words(h1, n1, h2, n2):
    # blocks 0..15: table base tiles; 16..21: irfft DFT matrices
    metaA = np.zeros((2, 22 * 128), np.float64)
    metaB = np.zeros((1, 22 * 128), np.float64)
    for sk, (h, neg) in ((0, (h1, n1)), (1, (h2, n2))):
        for var in range(2):          # 0 = re(cos), 1 = im(-sin)
            heff = h if var == 0 else (N - h) % N
            off = 4096 * neg + 4096 + (2048 if var == 0 else 0)
            for cc in range(4):
                blk = ((sk * 2 + var) * 4 + cc) * 128
                sl = slice(cc * 128, (cc + 1) * 128)
                metaA[0, blk:blk + 128] = heff[sl] & 63
                metaA[1, blk:blk + 128] = off[sl]
                metaB[0, blk:blk + 128] = heff[sl] >> 6
    p = np.arange(128)
    for i, (hq, hr, off) in enumerate((
        (p, 0 * p, 4096),          # e128s: sin(2pi*64*p*k/N)
        (p, 0 * p, 4096 + 2048),   # e128c
        (0 * p, p, 4096),          # tws:  sin(2pi*p*k/N)
        (0 * p, p, 4096 + 2048),   # twc
        (2 * p, 0 * p, 4096),      # e64s: sin(2pi*128*p*k/N)
        (2 * p, 0 * p, 4096 + 2048),  # e64c
    )):
        blk = (16 + i) * 128
        metaA[0, blk:blk + 128] = hr
        metaA[1, blk:blk + 128] = off
        metaB[0, blk:blk + 128] = hq
    return (metaA.astype(np.float16).view(np.float32),
            metaB.astype(np.float16).view(np.float32))


def make_in_maps(x, sketch1, sketch2):
    x = np.ascontiguousarray(np.asarray(x), dtype=np.float32).reshape(B, C, HW)
    h1, n1 = _extract(sketch1)
    h2, n2 = _extract(sketch2)
    tail = np.concatenate([_diag_words(h1, h2).ravel(),
                           _meta_words(h1, n1, h2, n2)[0].ravel(),
                           _meta_words(h1, n1, h2, n2)[1].ravel()])

    in_maps = []
    for i in range(NCORES):
        xp = np.empty(_XP_TOT, np.float32)
        xp[:_XP_DIAG] = x[i * BPC:(i + 1) * BPC].ravel()
        xp[_XP_DIAG:] = tail
        in_maps.append({"xpack": xp.reshape(1, -1)})
    return in_maps


def unshard_out(results):
    outs = np.empty((B, N), dtype=np.float32)
    for i in range(NCORES):
        z = np.asarray(results[i]["out"])          # int8 [BPC, 128, 68]
        for j in range(BPC):
            smax = z[j, 0, 64:68].copy().view(np.float32)[0]
            vals = z[j, :, 0:64].astype(np.float32) * (smax / 126.5)
            outs[i * BPC + j] = np.ascontiguousarray(vals.T).reshape(-1)
    return outs


# ---------------------------------------------------------------- device

def _build_program():
    import concourse.bass as bass
    import concourse.mybir as mybir
    import concourse.tile as tile
    from concourse import bacc

    f32 = mybir.dt.float32
    f16 = mybir.dt.float16
    i32 = mybir.dt.int32
    i8 = mybir.dt.int8
    AF = mybir.ActivationFunctionType
    OP = mybir.AluOpType

    nc = bacc.Bacc("TRN2", target_bir_lowering=False, debug=False,
                   num_devices=NCORES)

    xin = nc.dram_tensor("xpack", [1, _XP_TOT], f32, kind="ExternalInput").ap()
    out = nc.dram_tensor("out", [BPC, 128, 68], i8,
                         kind="ExternalOutput").ap()

    MW = (128, HW - 128)

    def xp_ap(off, pattern):
        return bass.AP(xin.tensor, off, pattern)

    with tile.TileContext(nc) as tc:
        with (
            tc.tile_pool(name="cpool", bufs=1) as cpool,
            tc.tile_pool(name="mpool", bufs=1) as mpool,
            tc.tile_pool(name="xpool", bufs=1) as xpool,
            tc.tile_pool(name="xstg", bufs=2) as xstg,
            tc.tile_pool(name="t0pool", bufs=1) as t0pool,
            tc.tile_pool(name="trot", bufs=2) as trot,
            tc.tile_pool(name="scr", bufs=1) as scr,
            tc.tile_pool(name="gwork", bufs=1) as gwork,
            tc.tile_pool(name="dpool", bufs=4) as dpool,
            tc.tile_pool(name="hpool", bufs=2) as hpool,
            tc.tile_pool(name="small", bufs=2) as small,
            tc.tile_pool(name="upsum", bufs=2, space="PSUM") as upsum,
            tc.tile_pool(name="gpsum", bufs=1, space="PSUM") as gpsum,
            tc.tile_pool(name="xpsum", bufs=1, space="PSUM") as xpsum,
            tc.tile_pool(name="spsum", bufs=1, space="PSUM") as spsum,
        ):
            # ---- small shipped inputs (from xpack) ----
            dgw = mpool.tile([128, 128], f32, tag="dgw", name="dgw")
            nc.sync.dma_start(dgw[:], xp_ap(_XP_DIAG, [[128, 128], [1, 128]]))
            diagv = mpool.tile([128, 256], f32, tag="diagv", name="diagv")
            nc.vector.tensor_copy(diagv[:], dgw[:].bitcast(f16))

            mAw = mpool.tile([2, 1408], f32, tag="mAw", name="mAw")
            nc.sync.dma_start(mAw[:], xp_ap(_XP_METAA, [[1408, 2], [1, 1408]]))
            metaA = mAw[:].bitcast(f16)          # [2, 2816] f16
            mBw = mpool.tile([1, 1408], f32, tag="mBw", name="mBw")
            nc.sync.dma_start(mBw[:], xp_ap(_XP_METAB, [[1408, 1], [1, 1408]]))
            metaB = mBw[:].bitcast(f16)          # [1, 2816] f16

            def dvec(sk, cc, t, which):
                c = ((sk * 4 + cc) * 8 + (t - 1)) * 4 + which
                return diagv[:, c:c + 1]

            # ---- device-built constants ----
            neg_pi = cpool.tile([128, 1], f32, tag="neg_pi", name="neg_pi")
            nc.gpsimd.memset(neg_pi[:], float(-np.pi))
            eps_b = cpool.tile([128, 1], f32, tag="eps_b", name="eps_b")
            nc.gpsimd.memset(eps_b[:], EPS_SQRT)
            eps_n = cpool.tile([128, 1], f32, tag="eps_n", name="eps_n")
            nc.gpsimd.memset(eps_n[:], float(N) * EPS_SQRT)

            band_t = cpool.tile([128, 127], f32, tag="band", name="band")
            nc.gpsimd.memset(band_t[:], 0.0)
            nc.gpsimd.memset(band_t[:, 63:64], 1.0)
            band = band_t[:]
            onesc_t = cpool.tile([128, 1], f32, tag="ones_col",
                                 name="ones_col")
            nc.gpsimd.memset(onesc_t[:], 1.0)
            ones_col = onesc_t[:]

            onesr = cpool.tile([1, 384], f32, tag="onesr", name="onesr")
            nc.gpsimd.memset(onesr[:, 0:128], 1.0)
            nc.gpsimd.memset(onesr[:, 128:256], -1.0)
            # alt row: (-1)^k = 1 - 2*(k & 1)
            alt_i = gwork.tile([1, 128], i32, tag="alt_i")
            nc.gpsimd.iota(alt_i[:], pattern=[[1, 128]], base=0,
                           channel_multiplier=0)
            nc.vector.tensor_scalar(alt_i[:], alt_i[:], 1, None,
                                    op0=OP.bitwise_and)
            alt_f = gwork.tile([1, 128], f32, tag="alt_f")
            nc.vector.tensor_copy(alt_f[:], alt_i[:])
            nc.vector.tensor_scalar(onesr[:, 256:384], alt_f[:], -2.0, 1.0,
                                    op0=OP.mult, op1=OP.add)
            ones_row = onesr[0:1, 0:128]
            mones_row = onesr[0:1, 128:256]
            alt_row = onesr[0:1, 256:384]

            # identity (f16) for diagonal-matmul construction
            ident = cpool.tile([128, 128], f16, tag="ident", name="ident")
            ident_io = gwork.tile([128, 128], i32, tag="ident_io")
            nc.gpsimd.iota(ident_io[:], pattern=[[1, 128]], base=0,
                           channel_multiplier=0)
            pidx = gwork.tile([128, 128], i32, tag="pidx")
            nc.gpsimd.iota(pidx[:], pattern=[[0, 128]], base=0,
                           channel_multiplier=1)
            nc.vector.tensor_tensor(ident[:], ident_io[:], pidx[:],
                                    op=OP.is_equal)

            # f16 iota/ones rows for the tiny integer matmuls
            # frows: row0 = 0..511, row1 = ones; built as j*(1-p) + p
            frows = cpool.tile([2, 512], f16, tag="frows", name="frows")
            fri = gwork.tile([2, 512], i32, tag="fri")
            nc.gpsimd.iota(fri[:], pattern=[[1, 512]], base=0,
                           channel_multiplier=0)
            pri = gwork.tile([2, 512], i32, tag="pri")
            nc.gpsimd.iota(pri[:], pattern=[[0, 512]], base=0,
                           channel_multiplier=1)
            fji = gwork.tile([2, 512], i32, tag="fji")
            nc.vector.tensor_tensor(fji[:], fri[:], pri[:], op=OP.mult)
            nc.vector.tensor_tensor(fji[:], fri[:], fji[:], op=OP.subtract)
            nc.vector.tensor_tensor(fji[:], fji[:], pri[:], op=OP.add)
            nc.vector.tensor_copy(frows[:], fji[:])

            # phase pipeline (shared by table base tiles and DFT mats):
            #   m1 = hq*f ; m2 = hr*f + off  (tiny f16 matmuls, exact ints)
            #   ph = ((int(m1) & 127) << 6) + int(m2) ; ph &= 8191
            #   out = sin(ph * 2pi/N - pi)
            def gen_phase(blk, rows, cols, out_ap):
                m1 = gpsum.tile([128, FT], f32, tag="gre", name="gre")
                nc.tensor.matmul(m1[:rows, :cols],
                                 metaB[0:1, blk:blk + rows],
                                 frows[0:1, 0:cols], start=True, stop=True)
                m2 = gpsum.tile([128, FT], f32, tag="gim", name="gim")
                nc.tensor.matmul(m2[:rows, :cols],
                                 metaA[:, blk:blk + rows],
                                 frows[:, 0:cols], start=True, stop=True)
                i1 = gwork.tile([128, FT], i32, tag="ph_i1",
                                name="ph_i1")[:rows, :cols]
                nc.vector.tensor_copy(i1, m1[:rows, :cols])
                nc.vector.tensor_scalar(i1, i1, 127, 6,
                                        op0=OP.bitwise_and,
                                        op1=OP.logical_shift_left)
                i2 = gwork.tile([128, FT], i32, tag="ph_i2",
                                name="ph_i2")[:rows, :cols]
                nc.vector.tensor_copy(i2, m2[:rows, :cols])
                nc.vector.tensor_tensor(i1, i1, i2, op=OP.add)
                nc.vector.tensor_scalar(i1, i1, 8191, None,
                                        op0=OP.bitwise_and)
                a = gwork.tile([128, FT], f32, tag="ph_a",
                               name="ph_a")[:rows, :cols]
                nc.vector.tensor_copy(a, i1)
                nc.scalar.activation(out_ap, a, AF.Sin,
                                     scale=float(2 * np.pi / N),
                                     bias=neg_pi[0:rows, :])

            # irfft DFT matrices via the phase pipeline (meta blocks 16..21)
            emats = {}
            for i, (nm, cols) in enumerate((
                ("e128s", 128), ("e128c", 128),
                ("tws", 128), ("twc", 128),
                ("e64s", 64), ("e64c", 64),
            )):
                em = cpool.tile([64, cols], f32, tag=nm, name=nm)
                gen_phase((16 + i) * 128, 64, cols, em[:])
                emats[nm] = em
            e128sn_t = cpool.tile([64, 128], f32, tag="e128sn", name="e128sn")
            nc.gpsimd.tensor_scalar(e128sn_t[:], emats["e128s"][:], -1.0,
                                    None, op0=OP.mult)
            e64sn_t = cpool.tile([64, 64], f32, tag="e64sn", name="e64sn")
            nc.gpsimd.tensor_scalar(e64sn_t[:], emats["e64s"][:], -1.0,
                                    None, op0=OP.mult)
            e128c, e128s = emats["e128c"][:], emats["e128s"][:]
            e128sn, e64sn = e128sn_t[:], e64sn_t[:]
            twc, tws = emats["twc"][:], emats["tws"][:]
            e64c = emats["e64c"][:]

            # ---- x load + fp16 pair split ----
            x16h = [[None] * 4 for _ in range(BPC)]
            x16l = [[None] * 4 for _ in range(BPC)]
            for b in range(BPC):
                for kc in range(4):
                    xt = xstg.tile([128, HW], f32, tag="xf32")
                    nc.sync.dma_start(
                        xt[:],
                        xp_ap(_XP_X + (b * C + kc * 128) * HW,
                              [[HW, 128], [1, HW]]))
                    xh = xpool.tile([128, HW], f16, tag=f"x16h_{b}_{kc}")
                    nc.vector.tensor_copy(xh[:], xt[:])
                    xl = xpool.tile([128, HW], f16, tag=f"x16l_{b}_{kc}")
                    nc.gpsimd.tensor_tensor(xl[:], xt[:], xh[:], op=OP.subtract)
                    x16h[b][kc] = xh
                    x16l[b][kc] = xl

            # ---- table generation ----
            # tile layout per (sk, cc, hi/lo): [128, 4chunk, 2var, 128] f16
            t0tab = {}      # (sk, cc, hl) -> persistent t=0 tile
            r0neg = {}      # (sk, cc, hl) -> negated contiguous re base
            curtab = {}     # (sk, cc, hl) -> current rotating tile (t>=1)

            def gen_base(sk, cc):
                for var in range(2):
                    blk = ((sk * 2 + var) * 4 + cc) * 128
                    s32 = gwork.tile([128, FT], f32, tag=f"s32_{var}",
                                     name=f"s32_{var}")
                    gen_phase(blk, 128, FT, s32[:])
                    th = t0tab[(sk, cc, 0)]
                    tl = t0tab[(sk, cc, 1)]
                    nc.scalar.copy(th[:, :, var, :],
                                   s32[:].rearrange("p (a b) -> p a b", a=4))
                    nc.gpsimd.tensor_tensor(
                        tl[:, :, var, :], s32[:].rearrange("p (a b) -> p a b", a=4),
                        th[:, :, var, :], op=OP.subtract)
                # negated re base (contiguous) for the im power-step
                rn_h = scr.tile([128, FT], f16, tag=f"rnh_{sk}_{cc}",
                                name=f"rnh_{sk}_{cc}")
                nc.gpsimd.tensor_scalar(rn_h[:].rearrange("p (a b) -> p a b", a=4),
                                        t0tab[(sk, cc, 0)][:, :, 0, :],
                                        -1.0, None, op0=OP.mult)
                rn_l = scr.tile([128, FT], f16, tag=f"rnl_{sk}_{cc}",
                                name=f"rnl_{sk}_{cc}")
                nc.gpsimd.tensor_scalar(rn_l[:].rearrange("p (a b) -> p a b", a=4),
                                        t0tab[(sk, cc, 1)][:, :, 0, :],
                                        -1.0, None, op0=OP.mult)
                r0neg[(sk, cc, 0)] = rn_h
                r0neg[(sk, cc, 1)] = rn_l

            def gen_power(sk, cc, t):
                # t == 8: only re/hi cols 4096..4159 are read (Nyquist)
                th0 = t0tab[(sk, cc, 0)]
                tl0 = t0tab[(sk, cc, 1)]
                if t == 8:
                    r0h = th0[:, 0, 0, 0:64]
                    r0l = tl0[:, 0, 0, 0:64]
                else:
                    r0h = th0[:, :, 0, :]
                    i0h = th0[:, :, 1, :]
                    r0l = tl0[:, :, 0, :]
                    i0l = tl0[:, :, 1, :]
                    rnh = r0neg[(sk, cc, 0)][:]
                    rnl = r0neg[(sk, cc, 1)][:]
                nd = 2 if t == 8 else 4
                dgs = []
                for which in range(nd):
                    d = dpool.tile([128, 128], f16, tag=f"d{which}",
                                   name=f"d{which}")
                    if which % 2 == 0:
                        nc.scalar.mul(d[:], ident[:], dvec(sk, cc, t, which))
                    else:
                        nc.gpsimd.tensor_scalar_mul(d[:], ident[:],
                                                    dvec(sk, cc, t, which))
                    dgs.append(d)
                th = trot.tile([128, 4, 2, CHUNK], f16, tag=f"tt_{sk}_{cc}_h",
                               name=f"tt_{sk}_{cc}_h")
                tl = trot.tile([128, 4, 2, CHUNK], f16, tag=f"tt_{sk}_{cc}_l",
                               name=f"tt_{sk}_{cc}_l")
                if t == 8:
                    drh, drl = dgs
                    pre = gpsum.tile([128, FT], f32, tag="gre", name="gre")
                    prw = pre[:, 0:64]
                    nc.tensor.matmul(prw, drh[:], r0h, start=True, stop=False)
                    nc.tensor.matmul(prw, drh[:], r0l, start=False,
                                     stop=False)
                    nc.tensor.matmul(prw, drl[:], r0h, start=False, stop=True)
                    nc.scalar.copy(th[:, 0, 0, 0:64], prw)
                    curtab[(sk, cc, 0)] = th
                    curtab[(sk, cc, 1)] = tl
                    return
                drh, drl, dnh, dnl = dgs
                pre = gpsum.tile([128, FT], f32, tag="gre", name="gre")
                nc.tensor.matmul(pre[:], drh[:], r0h, start=True, stop=False)
                nc.tensor.matmul(pre[:], drh[:], r0l, start=False, stop=False)
                nc.tensor.matmul(pre[:], drl[:], r0h, start=False, stop=False)
                nc.tensor.matmul(pre[:], dnh[:], i0h, start=False, stop=False)
                nc.tensor.matmul(pre[:], dnh[:], i0l, start=False, stop=False)
                nc.tensor.matmul(pre[:], dnl[:], i0h, start=False, stop=True)
                pim = gpsum.tile([128, FT], f32, tag="gim", name="gim")
                nc.tensor.matmul(pim[:], drh[:], i0h, start=True, stop=False)
                nc.tensor.matmul(pim[:], drh[:], i0l, start=False, stop=False)
                nc.tensor.matmul(pim[:], drl[:], i0h, start=False, stop=False)
                nc.tensor.matmul(pim[:], dnh[:], rnh, start=False, stop=False)
                nc.tensor.matmul(pim[:], dnh[:], rnl, start=False, stop=False)
                nc.tensor.matmul(pim[:], dnl[:], rnh, start=False, stop=True)
                for var, ps in ((0, pre), (1, pim)):
                    nc.scalar.copy(th[:, :, var, :],
                                   ps[:].rearrange("p (a b) -> p a b", a=4))
                    nc.vector.tensor_tensor(
                        tl[:, :, var, :], ps[:].rearrange("p (a b) -> p a b", a=4),
                        th[:, :, var, :], op=OP.subtract)
                curtab[(sk, cc, 0)] = th
                curtab[(sk, cc, 1)] = tl

            for sk in range(2):
                for cc in range(4):
                    t0tab[(sk, cc, 0)] = t0pool.tile(
                        [128, 4, 2, CHUNK], f16, tag=f"t0_{sk}_{cc}_h",
                        name=f"t0_{sk}_{cc}_h")
                    t0tab[(sk, cc, 1)] = t0pool.tile(
                        [128, 4, 2, CHUNK], f16, tag=f"t0_{sk}_{cc}_l",
                        name=f"t0_{sk}_{cc}_l")

            # ---- spectrum PSUM: 4 grids [64,64] + r16 [1,2] in one bank ----
            spect = xpsum.tile([64, 272], f32, tag="spect", name="spect")

            def xsp(b, p):
                return spect[0:64, 128 * b + 64 * p:128 * b + 64 * p + 64]

            r16 = spect[0:1, 256:256 + BPC]

            first_band = [True]

            def main_chunk_pair(chp, tabs):
                # chunks 2*chp, 2*chp+1 (512 table cols) at once: u1/u2 in
                # separate PSUM banks, column layout (re|im|re'|im')
                for b in range(BPC):
                    for mi, mw in enumerate(MW):
                        msl = bass.ds(mi * 128, mw)
                        u1t = upsum.tile([128, 4 * CHUNK], f32, tag="u1",
                                         name="u1t")
                        u2t = upsum.tile([128, 4 * CHUNK], f32, tag="u2",
                                         name="u2t")
                        ups = [u1t, u2t]
                        csl = bass.ds((chp & 1) * 2, 2)
                        for sk in range(2):
                            first = True
                            for kc in range(4):
                                lh = x16h[b][kc][:, msl]
                                ll = x16l[b][kc][:, msl]
                                rh = tabs[(sk, kc, 0)][:, csl, :, :]
                                rl = tabs[(sk, kc, 1)][:, csl, :, :]
                                for lhsT, rhs in ((lh, rh), (lh, rl),
                                                  (ll, rh)):
                                    nc.tensor.matmul(
                                        ups[sk][:mw], lhsT, rhs,
                                        start=first,
                                        stop=(kc == 3 and lhsT is ll))
                                    first = False
                        u2sb = hpool.tile([128, 4 * CHUNK], f32, tag="u2sb")
                        nc.scalar.copy(u2sb[:mw], ups[1][:mw])
                        u1v = ups[0][:mw].rearrange("p (a b) -> p a b", a=4)
                        u2v = u2sb[:mw].rearrange("p (a b) -> p a b", a=4)
                        u1r = u1v[:, 0::2, :]
                        u1i = u1v[:, 1::2, :]
                        u2r = u2v[:, 0::2, :]
                        u2i = u2v[:, 1::2, :]
                        t1 = hpool.tile([128, 2, CHUNK], f32, tag="t1")
                        t2 = hpool.tile([128, 2, CHUNK], f32, tag="t2")
                        t3 = hpool.tile([128, 2, CHUNK], f32, tag="t3")
                        t4 = hpool.tile([128, 2, CHUNK], f32, tag="t4")
                        h = hpool.tile([128, 4, CHUNK], f32, tag="h")
                        nc.vector.tensor_tensor(t1[:mw], u1r, u2r, op=OP.mult)
                        nc.vector.tensor_tensor(t2[:mw], u1i, u2i, op=OP.mult)
                        nc.gpsimd.tensor_tensor(h[:mw, 0::2, :], t1[:mw],
                                                t2[:mw], op=OP.subtract)
                        nc.vector.tensor_tensor(t3[:mw], u1r, u2i, op=OP.mult)
                        nc.vector.tensor_tensor(t4[:mw], u1i, u2r, op=OP.mult)
                        nc.gpsimd.tensor_tensor(h[:mw, 1::2, :],
                                                t3[:mw], t4[:mw], op=OP.add)
                        for lc in range(2):
                            ch = 2 * chp + lc
                            for r in range(2):
                                c = 2 * ch + r
                                lhsT = band[:mw, 63 - c:127 - c]
                                st = first_band[0]
                                first_band[0] = False
                                sp = (ch == NCHUNK - 1 and b == BPC - 1
                                      and mi == 1 and r == 1)
                                # one MM covers xsp(b,0)|xsp(b,1): out cols
                                # 128b..128b+128; rhs strided (re,im) pair
                                nc.tensor.matmul(
                                    spect[0:64, 128 * b:128 * b + 128],
                                    lhsT,
                                    h[:mw, 2 * lc:2 * lc + 2,
                                      64 * r:64 * r + 64],
                                    start=st, stop=sp,
                                    skip_group_check=True)

            def nyquist(tabs):
                # Re(Rhat[4096]) = sum_hw U1r*U2r at col 4096 (chunk 32 col 0)
                for b in range(BPC):
                    for mi, mw in enumerate(MW):
                        msl = bass.ds(mi * 128, mw)
                        u12 = upsum.tile([128, 4 * CHUNK], f32, tag="u1")
                        first = True
                        for kc in range(4):
                            lh = x16h[b][kc][:, msl]
                            for sk in range(2):
                                nc.tensor.matmul(
                                    u12[:mw, 64 * sk:64 * sk + 64],
                                    lh, tabs[(sk, kc, 0)][:, 0, 0, 0:64],
                                    start=first,
                                    stop=(kc == 3 and sk == 1))
                                first = False
                        u2sb = hpool.tile([128, 2 * CHUNK], f32, tag="u2sb")
                        nc.scalar.copy(u2sb[:mw, 0:1], u12[:mw, 64:65])
                        h = hpool.tile([128, 2 * CHUNK], f32, tag="h")
                        nc.vector.tensor_tensor(h[:mw, 0:1], u12[:mw, 0:1],
                                                u2sb[:mw, 0:1], op=OP.mult)
                        nc.tensor.matmul(r16[:, b:b + 1], ones_col[:mw, :],
                                         h[:mw, 0:1], start=False,
                                         stop=(b == BPC - 1 and mi == 1),
                                         skip_group_check=True)

            # ---- emission: pipelined table gen + main loop ----
            for sk in range(2):
                for cc in range(4):
                    gen_base(sk, cc)
            for t in range(0, NT):
                if t > 0:
                    for sk in range(2):
                        for cc in range(4):
                            gen_power(sk, cc, t)
                    tabs = dict(curtab)
                else:
                    tabs = dict(t0tab)
                for chp in range(2 * t, min(2 * t + 2, NCHUNK // 2)):
                    main_chunk_pair(chp, tabs)
                if t == 8:
                    nyquist(tabs)

            # ---- per batch: half-spectrum irfft + tail ----
            for b in range(BPC):
                xr = small.tile([64, 64], f32, tag="xr")
                xi = small.tile([64, 64], f32, tag="xi")
                nc.scalar.copy(xr[:], xsp(b, 0))
                nc.scalar.copy(xi[:], xsp(b, 1))
                r16_sb = small.tile([1, 1], f32, tag="r16_sb")
                nc.scalar.copy(r16_sb[:], r16[:, b:b + 1])

                sps = spsum.tile([128, 512], f32, tag="sps")
                yr = sps[0:64, 0:128]
                yi = sps[0:64, 128:256]
                zps = sps[0:128, 256:320]
                tot = sps[0:1, 320:321]
                nrmb = sps[0:128, 352:353]
                cps = sps[0:128, 384:385]

                nc.tensor.matmul(cps, mones_row, xr[0:1, 0:1],
                                 start=True, stop=False)
                nc.tensor.matmul(cps, alt_row, r16_sb[:], start=False,
                                 stop=True)
                c_sb = small.tile([128, 1], f32, tag="c_sb")
                nc.scalar.copy(c_sb[:], cps)

                nc.tensor.matmul(yr, xr[:], e128c, start=True, stop=False)
                nc.tensor.matmul(yr, xi[:], e128sn, start=False, stop=True)
                nc.tensor.matmul(yi, xr[:], e128s, start=True, stop=False)
                nc.tensor.matmul(yi, xi[:], e128c, start=False, stop=True)

                ypr = small.tile([64, 128], f32, tag="ypr")
                ypi = small.tile([64, 128], f32, tag="ypi")
                tt1 = small.tile([64, 128], f32, tag="tt1")
                tt2 = small.tile([64, 128], f32, tag="tt2")
                nc.vector.tensor_tensor(tt1[:], yr, twc, op=OP.mult)
                nc.vector.tensor_tensor(tt2[:], yi, tws, op=OP.mult)
                nc.vector.tensor_tensor(ypr[:], tt1[:], tt2[:],
                                        op=OP.subtract)
                nc.vector.tensor_tensor(tt1[:], yr, tws, op=OP.mult)
                nc.vector.tensor_tensor(tt2[:], yi, twc, op=OP.mult)
                nc.vector.tensor_tensor(ypi[:], tt1[:], tt2[:], op=OP.add)

                nc.tensor.matmul(zps, ypr[:], e64c, start=True, stop=False)
                nc.tensor.matmul(zps, ypi[:], e64sn, start=False, stop=True)

                zeff = small.tile([128, 64], f32, tag="zeff")
                nc.vector.tensor_scalar(zeff[:], zps, 2.0, c_sb[:, 0:1],
                                        op0=OP.mult, op1=OP.add)

                absz = small.tile([128, 64], f32, tag="absz")
                nc.scalar.activation(absz[:], zeff[:], AF.Abs)
                sq = small.tile([128, 64], f32, tag="sq")
                nc.scalar.activation(sq[:], absz[:], AF.Sqrt, bias=eps_b[:])
                sgn = small.tile([128, 64], f32, tag="sgn")
                nc.scalar.activation(sgn[:], zeff[:], AF.Sign)
                ssq = small.tile([128, 64], f32, tag="ssq")
                nc.vector.tensor_tensor(ssq[:], sq[:], sgn[:], op=OP.mult)
                rs = small.tile([128, 1], f32, tag="rs")
                nc.vector.reduce_sum(rs[:], zeff[:], axis=mybir.AxisListType.X,
                                     apply_absolute_value=True)
                nc.tensor.matmul(tot, rs[:], ones_col, start=True, stop=True)
                nrm = small.tile([1, 1], f32, tag="nrm")
                nc.scalar.activation(nrm[:], tot, AF.Sqrt, bias=eps_n[0:1, :])
                nc.vector.tensor_scalar_max(nrm[:], nrm[:], EPS_NORM)
                nc.vector.reciprocal(nrm[:], nrm[:])
                nc.tensor.matmul(nrmb, ones_row, nrm[:], start=True,
                                 stop=True)
                nrmb_s = small.tile([128, 1], f32, tag="nrmb_s")
                nc.scalar.copy(nrmb_s[:], nrmb)
                fin = small.tile([128, 64], f32, tag="fin")
                nc.vector.tensor_scalar_mul(fin[:], ssq[:], nrmb_s[:])
                # int8 quantization: q = round(fin * 127 / max|fin|)
                rmax = small.tile([128, 1], f32, tag="rmax")
                nc.vector.tensor_reduce(rmax[:], fin[:],
                                        axis=mybir.AxisListType.X,
                                        op=mybir.AluOpType.max,
                                        apply_absolute_value=True)
                rmaxT = small.tile([1, 128], f32, tag="rmaxT")
                nc.sync.dma_start(rmaxT[:], rmax[:])
                smax = small.tile([1, 1], f32, tag="smax")
                nc.vector.tensor_reduce(smax[:], rmaxT[:],
                                        axis=mybir.AxisListType.X,
                                        op=mybir.AluOpType.max)
                qs = small.tile([1, 1], f32, tag="qs")
                nc.vector.reciprocal(qs[:], smax[:])
                nc.vector.tensor_scalar(qs[:], qs[:], 126.5, None,
                                        op0=mybir.AluOpType.mult)
                qsb = spsum.tile([128, 512], f32, tag="sps",
                                 name="sps_q")[:, 480:481]
                nc.tensor.matmul(qsb, ones_row, qs[:], start=True, stop=True)
                qsb_s = small.tile([128, 1], f32, tag="qsb_s")
                nc.scalar.copy(qsb_s[:], qsb)
                qf = small.tile([128, 64], f32, tag="qf")
                nc.vector.tensor_scalar_mul(qf[:], fin[:], qsb_s[:])
                qv = small.tile([128, 64], i8, tag="qv")
                nc.vector.tensor_copy(qv[:], qf[:])
                nc.sync.dma_start(out[b, :, 0:64], qv[:])
                sc8 = small.tile([1, 4], i8, tag="sc8")
                nc.vector.tensor_copy(sc8[:], smax[:].bitcast(i8))
                nc.sync.dma_start(out[b, 0:1, 64:68], sc8[:])

    nc.compile()
    return nc


def _get_program():
    if "nc" not in _COMPILED:
        _COMPILED["nc"] = _build_program()
    return _COMPILED["nc"]


def kernel(x, sketch1, sketch2):
    from concourse.bass_utils import run_bass_kernel_spmd

    in_maps = make_in_maps(x, sketch1, sketch2)
    nc = _get_program()
    res = run_bass_kernel_spmd(nc, in_maps, core_ids=list(range(NCORES)))
    return unshard_out(res.results)



# revision 2
# speedup vs baseline: 1.5814x; 1.5814x over previous
"""Compact Bilinear Pooling (B=16, C=512, HW=196, OUT=8192) on 8 TRN2 cores.

v3: single-pass float32r matmuls (~13 mantissa bits) for the projection,
frequency tables regenerated per 512-column f-tile from an exact integer
phase pipeline:
  - one f32 matmul per (sk, var, cc) gives ph0 = heff*f' + off exactly,
  - per t-tile the offset (heff*512t mod 8192) is added on the Act engine
    (Identity + per-partition bias, u16), masked (&8191) on DVE/Pool, and
    turned into table values by one Sin activation per (sk, cc, var).
x ships as int16 (the global scale cancels in the final L2 normalize).
The complex product writes four sign-prepared product tiles (u2 imag
negated once) so the spatial sum runs as pure-accumulate f32r band matmuls
with 256-column outputs (1 cycle/row). Output: int8 with per-batch scale.

Sharding: data-parallel over batch, 2 batches per core, no collectives.
"""

import numpy as np

B, C, HW, N = 16, 512, 196, 8192
NF = N // 2 + 1
CHUNK = 128              # frequency columns per chunk
FT = 512                 # frequency columns per generated f-tile
NT = 8                   # full f-tiles used by the main loop (chunks 0..31)
NCHUNK = 32
NCORES = 8
BPC = B // NCORES
EPS_SQRT = 1e-5
EPS_NORM = 1e-12

_COMPILED = {}

# ---------------------------------------------------------------- host side

# xpack int16 layout (units: int16 words):
#   x (BPC*C*HW) | metaA f32 [2, 2816] (as int16 pairs)
_XP_X = 0
_XP_META = BPC * C * HW
_XP_TOT = _XP_META + 2 * 2816 * 2


def _extract(sk):
    sk = np.asarray(sk)
    h = np.abs(sk).argmax(axis=1).astype(np.int64)
    s = sk[np.arange(C), h]
    return h, (s < 0).astype(np.int64)


def _meta_words(h1, n1, h2, n2):
    # blocks 0..15: (sk, var, cc) table phases; 16..21: irfft DFT matrices
    # row 0 = heff, row 1 = off; phase = heff*f + off (exact in f32)
    metaA = np.zeros((2, 22 * 128), np.float32)
    for sk, (h, neg) in ((0, (h1, n1)), (1, (h2, n2))):
        for var in range(2):          # 0 = re(cos), 1 = im(-sin)
            heff = h if var == 0 else (N - h) % N
            off = 4096 * neg + 4096 + (2048 if var == 0 else 0)
            for cc in range(4):
                blk = ((sk * 2 + var) * 4 + cc) * 128
                sl = slice(cc * 128, (cc + 1) * 128)
                metaA[0, blk:blk + 128] = heff[sl]
                metaA[1, blk:blk + 128] = off[sl]
    p = np.arange(128)
    for i, (hval, off) in enumerate((
        (64 * p, 4096),          # e128s: sin(2pi*64*p*k/N)
        (64 * p, 4096 + 2048),   # e128c
        (p, 4096),               # tws:  sin(2pi*p*k/N)
        (p, 4096 + 2048),        # twc
        (128 * p, 4096),         # e64s: sin(2pi*128*p*k/N)
        (128 * p, 4096 + 2048),  # e64c
    )):
        blk = (16 + i) * 128
        metaA[0, blk:blk + 128] = hval
        metaA[1, blk:blk + 128] = off
    return metaA


def make_in_maps(x, sketch1, sketch2):
    x = np.ascontiguousarray(np.asarray(x), dtype=np.float32).reshape(B, C, HW)
    s = float(np.abs(x).max()) / 32767.0
    xq = np.clip(np.round(x / s), -32767, 32767).astype(np.int16)
    h1, n1 = _extract(sketch1)
    h2, n2 = _extract(sketch2)
    tail = _meta_words(h1, n1, h2, n2).view(np.int16).ravel()

    in_maps = []
    for i in range(NCORES):
        xp = np.empty(_XP_TOT, np.int16)
        xp[:_XP_META] = xq[i * BPC:(i + 1) * BPC].ravel()
        xp[_XP_META:] = tail
        in_maps.append({"xpack": xp.reshape(1, -1)})
    return in_maps


def unshard_out(results):
    outs = np.empty((B, N), dtype=np.float32)
    for i in range(NCORES):
        z = np.asarray(results[i]["out"])          # int8 [BPC, 128, 68]
        for j in range(BPC):
            smax = z[j, 0, 64:68].copy().view(np.float32)[0]
            vals = z[j, :, 0:64].astype(np.float32) * (smax / 126.5)
            outs[i * BPC + j] = np.ascontiguousarray(vals.T).reshape(-1)
    return outs


# ---------------------------------------------------------------- device

def _build_program():
    import concourse.bass as bass
    import concourse.mybir as mybir
    import concourse.tile as tile
    from concourse import bacc

    f32 = mybir.dt.float32
    f32r = mybir.dt.float32r
    i32 = mybir.dt.int32
    i16 = mybir.dt.int16
    u16 = mybir.dt.uint16
    i8 = mybir.dt.int8
    AF = mybir.ActivationFunctionType
    OP = mybir.AluOpType

    nc = bacc.Bacc("TRN2", target_bir_lowering=False, debug=False,
                   num_devices=NCORES)

    xin = nc.dram_tensor("xpack", [1, _XP_TOT], i16, kind="ExternalInput").ap()
    out = nc.dram_tensor("out", [BPC, 128, 68], i8,
                         kind="ExternalOutput").ap()

    MW = (128, HW - 128)
    SIN_SC = float(2 * np.pi / N)

    def xp_ap(off, pattern):
        return bass.AP(xin.tensor, off, pattern)

    with tile.TileContext(nc) as tc:
        with (
            tc.tile_pool(name="cpool", bufs=1) as cpool,
            tc.tile_pool(name="mpool", bufs=1) as mpool,
            tc.tile_pool(name="xpool", bufs=1) as xpool,
            tc.tile_pool(name="xstg", bufs=2) as xstg,
            tc.tile_pool(name="ipool", bufs=1) as ipool,
            tc.tile_pool(name="gwork", bufs=3) as gwork,
            tc.tile_pool(name="trot", bufs=2) as trot,
            tc.tile_pool(name="hpool", bufs=2) as hpool,
            tc.tile_pool(name="small", bufs=2) as small,
            tc.tile_pool(name="upsA", bufs=2, space="PSUM") as upsA,
            tc.tile_pool(name="upsB", bufs=1, space="PSUM") as upsB,
            tc.tile_pool(name="gpsum", bufs=1, space="PSUM") as gpsum,
            tc.tile_pool(name="xpsum", bufs=1, space="PSUM") as xpsum,
        ):
            # ---- shipped meta: f32 [2, 2816] (heff row, off row) ----
            mAw = mpool.tile([2, 5632], i16, tag="mAw", name="mAw")
            nc.sync.dma_start(mAw[:], xp_ap(_XP_META, [[5632, 2], [1, 5632]]))
            metaAF = mAw[:].bitcast(f32)

            # ---- device-built constants ----
            neg_pi = cpool.tile([128, 1], f32, tag="neg_pi", name="neg_pi")
            nc.gpsimd.memset(neg_pi[:], float(-np.pi))
            eps_b = cpool.tile([128, 1], f32, tag="eps_b", name="eps_b")
            nc.gpsimd.memset(eps_b[:], EPS_SQRT)
            eps_n = cpool.tile([128, 1], f32, tag="eps_n", name="eps_n")
            nc.gpsimd.memset(eps_n[:], float(N) * EPS_SQRT)

            band_f = cpool.tile([128, 127], f32, tag="band_f", name="band_f")
            nc.gpsimd.memset(band_f[:], 0.0)
            nc.gpsimd.memset(band_f[:, 63:64], 1.0)
            band_t = cpool.tile([128, 127], f32r, tag="band", name="band")
            nc.vector.tensor_copy(band_t[:], band_f[:])
            band = band_t[:]
            onesc_t = cpool.tile([128, 1], f32, tag="ones_col",
                                 name="ones_col")
            nc.gpsimd.memset(onesc_t[:], 1.0)
            ones_col = onesc_t[:]

            onesr = cpool.tile([1, 384], f32, tag="onesr", name="onesr")
            nc.gpsimd.memset(onesr[:, 0:128], 1.0)
            nc.gpsimd.memset(onesr[:, 128:256], -1.0)
            alt_i = gwork.tile([1, 128], i32, tag="alt_i")
            nc.gpsimd.iota(alt_i[:], pattern=[[1, 128]], base=0,
                           channel_multiplier=0)
            nc.vector.tensor_scalar(alt_i[:], alt_i[:], 1, None,
                                    op0=OP.bitwise_and)
            alt_f = gwork.tile([1, 128], f32, tag="alt_f")
            nc.vector.tensor_copy(alt_f[:], alt_i[:])
            nc.vector.tensor_scalar(onesr[:, 256:384], alt_f[:], -2.0, 1.0,
                                    op0=OP.mult, op1=OP.add)
            ones_row = onesr[0:1, 0:128]
            mones_row = onesr[0:1, 128:256]
            alt_row = onesr[0:1, 256:384]

            # f32 iota/ones rows for the phase matmuls:
            # row0 = 0..511, row1 = ones; built as j*(1-p) + p
            frows = cpool.tile([2, 512], f32, tag="frows", name="frows")
            fri = gwork.tile([2, 512], i32, tag="fri")
            nc.gpsimd.iota(fri[:], pattern=[[1, 512]], base=0,
                           channel_multiplier=0)
            pri = gwork.tile([2, 512], i32, tag="pri")
            nc.gpsimd.iota(pri[:], pattern=[[0, 512]], base=0,
                           channel_multiplier=1)
            fji = gwork.tile([2, 512], i32, tag="fji")
            nc.vector.tensor_tensor(fji[:], fri[:], pri[:], op=OP.mult)
            nc.vector.tensor_tensor(fji[:], fri[:], fji[:], op=OP.subtract)
            nc.vector.tensor_tensor(fji[:], fji[:], pri[:], op=OP.add)
            nc.vector.tensor_copy(frows[:], fji[:])

            # one matmul: m[c, f'] = heff[c]*f' + off[c]  (exact in f32)
            _phm_flip = [0]

            def phase_mm(blk, rows, cols):
                if _phm_flip[0] % 2 == 0:
                    m = gpsum.tile([128, FT], f32, tag="gre", name="gre")
                else:
                    m = upsB.tile([128, FT], f32, tag="u1b1", name="grb")
                _phm_flip[0] += 1
                nc.tensor.matmul(m[:rows, :cols],
                                 metaAF[:, blk:blk + rows],
                                 frows[:, 0:cols], start=True, stop=True)
                return m

            # full f32 phase pipeline (DFT matrices only)
            def gen_phase(blk, rows, cols, out_ap):
                m = phase_mm(blk, rows, cols)
                i1 = gwork.tile([128, FT], i32, tag="ph_i1",
                                name="ph_i1")[:rows, :cols]
                nc.vector.tensor_copy(i1, m[:rows, :cols])
                nc.vector.tensor_scalar(i1, i1, 8191, None,
                                        op0=OP.bitwise_and)
                a = gwork.tile([128, FT], f32, tag="ph_a",
                               name="ph_a")[:rows, :cols]
                nc.vector.tensor_copy(a, i1)
                nc.scalar.activation(out_ap, a, AF.Sin, scale=SIN_SC,
                                     bias=neg_pi[0:rows, :])

            # irfft DFT matrices
            emats = {}
            for i, (nm, cols) in enumerate((
                ("e128s", 128), ("e128c", 128),
                ("tws", 128), ("twc", 128),
                ("e64s", 64), ("e64c", 64),
            )):
                em = cpool.tile([64, cols], f32, tag=nm, name=nm)
                gen_phase((16 + i) * 128, 64, cols, em[:])
                emats[nm] = em
            twc, tws = emats["twc"][:], emats["tws"][:]
            e64c = emats["e64c"][:]
            e64sn_t = cpool.tile([64, 64], f32, tag="e64sn", name="e64sn")
            nc.gpsimd.tensor_scalar(e64sn_t[:], emats["e64s"][:], -1.0,
                                    None, op0=OP.mult)
            e64sn = e64sn_t[:]
            # merged f32r DFT tiles: e_yr = [e128c | e128s],
            # e_yi = [-e128s | e128c]
            e_yr_t = cpool.tile([64, 256], f32r, tag="e_yr", name="e_yr")
            nc.vector.tensor_copy(e_yr_t[:, 0:128], emats["e128c"][:])
            nc.vector.tensor_copy(e_yr_t[:, 128:256], emats["e128s"][:])
            e_yi_t = cpool.tile([64, 256], f32r, tag="e_yi", name="e_yi")
            nc.vector.tensor_scalar(e_yi_t[:, 0:128], emats["e128s"][:],
                                    -1.0, None, op0=OP.mult)
            nc.vector.tensor_copy(e_yi_t[:, 128:256], emats["e128c"][:])

            # ---- x load: int16 -> f32r (scale folds out) ----
            xr32 = [[None] * 4 for _ in range(BPC)]
            for b in range(BPC):
                for kc in range(4):
                    xt = xstg.tile([128, HW], i16, tag="xi16")
                    nc.sync.dma_start(
                        xt[:],
                        xp_ap(_XP_X + (b * C + kc * 128) * HW,
                              [[HW, 128], [1, HW]]))
                    xr = xpool.tile([128, HW], f32r, tag=f"xr_{b}_{kc}")
                    nc.vector.tensor_copy(xr[:], xt[:])
                    xr32[b][kc] = xr

            # ---- integer phase bases q0 (u16, premasked) + per-t f32
            # offsets qd (for the Act-engine add) ----
            q0s = {}
            qds = {}
            for sk in range(2):
                for cc in range(4):
                    for var in range(2):
                        blk = ((sk * 2 + var) * 4 + cc) * 128
                        m = phase_mm(blk, 128, FT)
                        i1 = gwork.tile([128, FT], i32, tag="ph_i1",
                                        name="ph_i1")
                        nc.vector.tensor_copy(i1[:], m[:])
                        # heff per partition = m[:,1] - m[:,0]
                        hcol = gwork.tile([128, 1], i32, tag="hcol")
                        nc.vector.tensor_tensor(hcol[:], i1[:, 1:2],
                                                i1[:, 0:1], op=OP.subtract)
                        nc.vector.tensor_scalar(i1[:], i1[:], 8191, None,
                                                op0=OP.bitwise_and)
                        q0 = ipool.tile([128, FT], u16,
                                        tag=f"q0_{sk}_{cc}_{var}")
                        nc.scalar.copy(q0[:], i1[:])
                        q0s[(sk, cc, var)] = q0
                        hdm = gwork.tile([128, NT + 1], i32, tag="hdm")
                        for t in range(NT + 1):
                            nc.vector.tensor_scalar(
                                hdm[:, t:t + 1], hcol[:], 512 * t, None,
                                op0=OP.mult)
                        nc.vector.tensor_scalar(hdm[:], hdm[:], 8191, None,
                                                op0=OP.bitwise_and)
                        qd = ipool.tile([128, NT + 1], f32,
                                        tag=f"qd_{sk}_{cc}_{var}")
                        nc.vector.tensor_copy(qd[:], hdm[:])
                        qds[(sk, cc, var)] = qd

            # ---- per-t table generation: act-add + mask + sin ----
            def gen_tile(t):
                tabs = {}
                for sk in range(2):
                    for cc in range(4):
                        tt = trot.tile([128, 4, 2, CHUNK], f32r,
                                       tag=f"tt_{sk}_{cc}",
                                       name=f"tt_{sk}_{cc}")
                        for var in range(2):
                            if t == 0:
                                qv = q0s[(sk, cc, var)]
                            else:
                                sq = gwork.tile([128, FT], u16,
                                                tag=f"sq_{var}", name="sq")
                                nc.gpsimd.tensor_scalar(
                                    sq[:], q0s[(sk, cc, var)][:],
                                    qds[(sk, cc, var)][:, t:t + 1],
                                    None, op0=OP.add)
                                qv = gwork.tile([128, FT], u16,
                                                tag=f"qv_{var}", name="qv")
                                nc.vector.tensor_scalar(qv[:], sq[:], 8191,
                                                        None,
                                                        op0=OP.bitwise_and)
                            nc.scalar.activation(
                                tt[:, :, var, :],
                                qv[:].rearrange("p (a b) -> p a b", a=4),
                                AF.Sin, scale=SIN_SC, bias=neg_pi[:])
                        tabs[(sk, cc)] = tt
                return tabs

            # t=8 tile: Nyquist cos columns only (f' 0..63, var 0)
            def gen_tile8():
                tabs8 = {}
                for sk in range(2):
                    for cc in range(4):
                        tt = trot.tile([128, 64], f32r, tag=f"t8_{sk}_{cc}",
                                       name=f"t8_{sk}_{cc}")
                        sq = gwork.tile([128, FT], u16, tag="sq_0",
                                        name="sq")[:, 0:64]
                        nc.gpsimd.tensor_scalar(
                            sq, q0s[(sk, cc, 0)][:, 0:64],
                            qds[(sk, cc, 0)][:, 8:9], None, op0=OP.add)
                        qv = gwork.tile([128, FT], u16, tag="qv_0",
                                        name="qv")[:, 0:64]
                        nc.vector.tensor_scalar(qv, sq, 8191, None,
                                                op0=OP.bitwise_and)
                        nc.scalar.activation(tt[:], qv, AF.Sin,
                                             scale=SIN_SC, bias=neg_pi[:])
                        tabs8[(sk, cc)] = tt
                return tabs8

            # ---- spectrum PSUM: 4 grids [64,64] (2 batches) + r16 ----
            spect = xpsum.tile([64, 272], f32, tag="spect", name="spect")

            def xsp(b, p):
                return spect[0:64, 128 * b + 64 * p:128 * b + 64 * p + 64]

            r16 = spect[0:1, 256:256 + BPC]

            first_band = [True]
            pending_band = []

            def flush_band():
                for fn in pending_band:
                    fn()
                pending_band.clear()

            def main_chunk_pair(chp, tabs):
                csl = bass.ds((chp & 1) * 2, 2)
                for mi, mw in enumerate(MW):
                    msl = bass.ds(mi * 128, mw)
                    ups = {}
                    for b in range(BPC):
                        upool = upsA if b == 0 else upsB
                        u1 = upool.tile([128, 4 * CHUNK], f32,
                                        tag=f"u1b{b}", name=f"u1b{b}")
                        u2 = upool.tile([128, 4 * CHUNK], f32,
                                        tag=f"u2b{b}", name=f"u2b{b}")
                        for sk, ut in ((0, u1), (1, u2)):
                            for kc in range(4):
                                nc.tensor.matmul(
                                    ut[:mw], xr32[b][kc][:, msl],
                                    tabs[(sk, kc)][:, csl, :, :],
                                    start=(kc == 0), stop=(kc == 3))
                        ups[b] = (u1, u2)
                    # previous iteration's band matmuls go behind this
                    # iteration's u-matmuls so PE never waits on products
                    flush_band()
                    # product tiles, band-ready layout [p, b, lc, var, q]:
                    # tcat1 = (u1r*u2r | u1r*u2i), tcat2 = (-u1i*u2i | u1i*u2r)
                    tcat1 = hpool.tile([128, 2, 2, 2, CHUNK], f32r,
                                       tag="tcat1")
                    tcat2 = hpool.tile([128, 2, 2, 2, CHUNK], f32r,
                                       tag="tcat2")
                    for b in range(BPC):
                        u1, u2 = ups[b]
                        u2sb = hpool.tile([128, 4 * CHUNK], f32, tag="u2sb")
                        nc.scalar.copy(u2sb[:mw], u2[:mw])
                        u1v = u1[:mw].rearrange("p (a b) -> p a b", a=4)
                        u2v = u2sb[:mw].rearrange("p (a b) -> p a b", a=4)
                        u1r = u1v[:, 0::2, :]
                        u1i = u1v[:, 1::2, :]
                        u2r = u2v[:, 0::2, :]
                        u2i = u2v[:, 1::2, :]
                        u2in = hpool.tile([128, 2, CHUNK], f32, tag="u2in")
                        nc.gpsimd.tensor_scalar(u2in[:mw], u2i, -1.0, None,
                                                op0=OP.mult)
                        nc.vector.tensor_tensor(tcat1[:mw, b, :, 0, :],
                                                u1r, u2r, op=OP.mult)
                        nc.vector.tensor_tensor(tcat1[:mw, b, :, 1, :],
                                                u1r, u2i, op=OP.mult)
                        nc.vector.tensor_tensor(tcat2[:mw, b, :, 0, :],
                                                u1i, u2in[:mw], op=OP.mult)
                        nc.vector.tensor_tensor(tcat2[:mw, b, :, 1, :],
                                                u1i, u2r, op=OP.mult)

                    def emit_band(mw=mw, chp=chp, mi=mi, tcat1=tcat1,
                                  tcat2=tcat2):
                        for lc in range(2):
                            ch = 2 * chp + lc
                            for r in range(2):
                                c = 2 * ch + r
                                st = first_band[0]
                                first_band[0] = False
                                sp = (ch == NCHUNK - 1 and mi == 1
                                      and r == 1)
                                nc.tensor.matmul(
                                    spect[0:64, 0:256],
                                    band[:mw, 63 - c:127 - c],
                                    tcat1[:mw, :, lc, :, 64 * r:64 * r + 64],
                                    start=st, stop=False,
                                    skip_group_check=True)
                                nc.tensor.matmul(
                                    spect[0:64, 0:256],
                                    band[:mw, 63 - c:127 - c],
                                    tcat2[:mw, :, lc, :, 64 * r:64 * r + 64],
                                    start=False, stop=sp,
                                    skip_group_check=True)

                    pending_band.append(emit_band)

            def nyquist(tabs8):
                # Re(Rhat[4096]) = sum_hw U1[4096]*U2[4096]
                for b in range(BPC):
                    for mi, mw in enumerate(MW):
                        msl = bass.ds(mi * 128, mw)
                        u12 = upsA.tile([128, 4 * CHUNK], f32, tag="u1b0",
                                        name="u1b0")
                        for sk in range(2):
                            for kc in range(4):
                                nc.tensor.matmul(
                                    u12[:mw, 64 * sk:64 * sk + 64],
                                    xr32[b][kc][:, msl],
                                    tabs8[(sk, kc)][:],
                                    start=(kc == 0), stop=(kc == 3))
                        h = hpool.tile([128, 2 * CHUNK], f32, tag="hny")
                        nc.scalar.copy(h[:mw, 1:2], u12[:mw, 64:65])
                        nc.vector.tensor_tensor(h[:mw, 0:1], u12[:mw, 0:1],
                                                h[:mw, 1:2], op=OP.mult)
                        nc.tensor.matmul(r16[:, b:b + 1], ones_col[:mw, :],
                                         h[:mw, 0:1],
                                         start=(mi == 0),
                                         stop=(b == BPC - 1 and mi == 1),
                                         skip_group_check=True)

            # ---- emission: pipelined table gen + main loop ----
            for t in range(NT):
                tabs = gen_tile(t)
                for chp in range(2 * t, 2 * t + 2):
                    main_chunk_pair(chp, tabs)
            tabs8 = gen_tile8()
            flush_band()
            nyquist(tabs8)

            # ---- per batch: half-spectrum irfft + tail ----
            # ops emitted interleaved across the two batches so each
            # engine's in-order stream can overlap the two chains
            TL = {}

            def step(fn):
                for b in range(BPC):
                    fn(b)

            def s_tile(b, shape, dt_, tag):
                key = (tag, b)
                if key not in TL:
                    TL[key] = small.tile(shape, dt_, tag=f"{tag}_{b}",
                                         name=f"{tag}_{b}")
                return TL[key]

            sps_t = {}
            sps_t[0] = gpsum.tile([128, 512], f32, tag="gre", name="sps_0")
            sps_t[1] = upsB.tile([128, 512], f32, tag="u1b1", name="sps_1")

            def st_copyspect(b):
                xr = s_tile(b, [64, 64], f32r, "xr")
                xi = s_tile(b, [64, 64], f32r, "xi")
                nc.vector.tensor_copy(xr[:], xsp(b, 0))
                nc.vector.tensor_copy(xi[:], xsp(b, 1))
                xr00 = s_tile(b, [1, 1], f32, "xr00")
                nc.scalar.copy(xr00[:], spect[0:1, 128 * b:128 * b + 1])
                r16_sb = s_tile(b, [1, 1], f32, "r16_sb")
                nc.scalar.copy(r16_sb[:], r16[:, b:b + 1])
            step(st_copyspect)

            def st_cps(b):
                sps = sps_t[b]
                cps = sps[0:128, 384:385]
                nc.tensor.matmul(cps, mones_row, TL[("xr00", b)][:],
                                 start=True, stop=False)
                nc.tensor.matmul(cps, alt_row, TL[("r16_sb", b)][:],
                                 start=False, stop=True)
                c_sb = s_tile(b, [128, 1], f32, "c_sb")
                nc.scalar.copy(c_sb[:], cps)
                nc.tensor.matmul(sps[0:64, 0:256], TL[("xr", b)][:],
                                 e_yr_t[:], start=True, stop=False)
                nc.tensor.matmul(sps[0:64, 0:256], TL[("xi", b)][:],
                                 e_yi_t[:], start=False, stop=True)
            step(st_cps)

            def st_tw(b):
                sps = sps_t[b]
                yr = sps[0:64, 0:128]
                yi = sps[0:64, 128:256]
                ypr = s_tile(b, [64, 128], f32, "ypr")
                ypi = s_tile(b, [64, 128], f32, "ypi")
                tt1 = s_tile(b, [64, 128], f32, "tt1")
                tt2 = s_tile(b, [64, 128], f32, "tt2")
                nc.vector.tensor_tensor(tt1[:], yr, twc, op=OP.mult)
                nc.vector.tensor_tensor(tt2[:], yi, tws, op=OP.mult)
                nc.vector.tensor_tensor(ypr[:], tt1[:], tt2[:],
                                        op=OP.subtract)
                nc.vector.tensor_tensor(tt1[:], yr, tws, op=OP.mult)
                nc.vector.tensor_tensor(tt2[:], yi, twc, op=OP.mult)
                nc.vector.tensor_tensor(ypi[:], tt1[:], tt2[:], op=OP.add)
            step(st_tw)

            def st_z(b):
                sps = sps_t[b]
                zps = sps[0:128, 256:320]
                nc.tensor.matmul(zps, TL[("ypr", b)][:], e64c, start=True,
                                 stop=False)
                nc.tensor.matmul(zps, TL[("ypi", b)][:], e64sn, start=False,
                                 stop=True)
                zeff = s_tile(b, [128, 64], f32, "zeff")
                nc.vector.tensor_scalar(zeff[:], zps, 2.0,
                                        TL[("c_sb", b)][:, 0:1],
                                        op0=OP.mult, op1=OP.add)
            step(st_z)

            def st_ssq(b):
                zeff = TL[("zeff", b)]
                absz = s_tile(b, [128, 64], f32, "absz")
                nc.scalar.activation(absz[:], zeff[:], AF.Abs)
                sq = s_tile(b, [128, 64], f32, "sq")
                nc.scalar.activation(sq[:], absz[:], AF.Sqrt, bias=eps_b[:])
                sgn = s_tile(b, [128, 64], f32, "sgn")
                nc.scalar.activation(sgn[:], zeff[:], AF.Sign)
                ssq = s_tile(b, [128, 64], f32, "ssq")
                nc.vector.tensor_tensor(ssq[:], sq[:], sgn[:], op=OP.mult)
                rs = s_tile(b, [128, 1], f32, "rs")
                nc.vector.reduce_sum(rs[:], zeff[:],
                                     axis=mybir.AxisListType.X,
                                     apply_absolute_value=True)
            step(st_ssq)

            def st_nrm(b):
                sps = sps_t[b]
                tot = sps[0:1, 320:321]
                nrmb = sps[0:128, 352:353]
                nc.tensor.matmul(tot, TL[("rs", b)][:], ones_col,
                                 start=True, stop=True)
                nrm = s_tile(b, [1, 1], f32, "nrm")
                nc.scalar.activation(nrm[:], tot, AF.Sqrt,
                                     bias=eps_n[0:1, :])
                nc.vector.tensor_scalar_max(nrm[:], nrm[:], EPS_NORM)
                nc.vector.reciprocal(nrm[:], nrm[:])
                nc.tensor.matmul(nrmb, ones_row, nrm[:], start=True,
                                 stop=True)
                nrmb_s = s_tile(b, [128, 1], f32, "nrmb_s")
                nc.scalar.copy(nrmb_s[:], nrmb)
                fin = s_tile(b, [128, 64], f32, "fin")
                nc.vector.tensor_scalar_mul(fin[:], TL[("ssq", b)][:],
                                            nrmb_s[:])
            step(st_nrm)

            def st_qmax(b):
                fin = TL[("fin", b)]
                rmax = s_tile(b, [128, 1], f32, "rmax")
                nc.vector.tensor_reduce(rmax[:], fin[:],
                                        axis=mybir.AxisListType.X,
                                        op=mybir.AluOpType.max,
                                        apply_absolute_value=True)
                rmaxT = s_tile(b, [1, 128], f32, "rmaxT")
                nc.sync.dma_start(rmaxT[:], rmax[:])
                smax = s_tile(b, [1, 1], f32, "smax")
                nc.vector.tensor_reduce(smax[:], rmaxT[:],
                                        axis=mybir.AxisListType.X,
                                        op=mybir.AluOpType.max)
                qs = s_tile(b, [1, 1], f32, "qs")
                nc.vector.reciprocal(qs[:], smax[:])
                nc.vector.tensor_scalar(qs[:], qs[:], 126.5, None,
                                        op0=mybir.AluOpType.mult)
            step(st_qmax)

            def st_qout(b):
                sps = sps_t[b]
                qsb = sps[:, 480:481]
                nc.tensor.matmul(qsb, ones_row, TL[("qs", b)][:],
                                 start=True, stop=True)
                qsb_s = s_tile(b, [128, 1], f32, "qsb_s")
                nc.scalar.copy(qsb_s[:], qsb)
                qf = s_tile(b, [128, 64], f32, "qf")
                nc.vector.tensor_scalar_mul(qf[:], TL[("fin", b)][:],
                                            qsb_s[:])
                qv = s_tile(b, [128, 64], i8, "qv")
                nc.vector.tensor_copy(qv[:], qf[:])
                nc.sync.dma_start(out[b, :, 0:64], qv[:])
                sc8 = s_tile(b, [1, 4], i8, "sc8")
                nc.vector.tensor_copy(sc8[:], TL[("smax", b)][:].bitcast(i8))
                nc.sync.dma_start(out[b, 0:1, 64:68], sc8[:])
            step(st_qout)

    nc.compile()
    return nc


def _get_program():
    if "nc" not in _COMPILED:
        _COMPILED["nc"] = _build_program()
    return _COMPILED["nc"]


def kernel(x, sketch1, sketch2):
    from concourse.bass_utils import run_bass_kernel_spmd

    in_maps = make_in_maps(x, sketch1, sketch2)
    nc = _get_program()
    res = run_bass_kernel_spmd(nc, in_maps, core_ids=list(range(NCORES)))
    return unshard_out(res.results)
